# revision 1
# baseline (speedup 1.0000x reference)
"""Trainium2 Bass kernel for nn_CascadeGNN (cascade AGNN over 256 graphs).

Strategy (graph-sharded SPMD over 8 NeuronCores, 32 graphs/core):
  * All AGNN message passing is done densely per graph: edges within a graph
    are encoded as a dense [512,512] multiplicity (count) matrix Ct built on
    host from the int32 edge lists (pure topology/format conversion; all
    float compute runs on device).
  * AGNN without max-subtraction (softmax is shift-invariant; beta*cos is
    bounded), so per graph:
        cos  = hn^T hn + c * ir ir^T      (rank-1 term absorbs the per-graph
                                           broadcast query vector; c=|u_g|^2)
        W    = Ct * exp(beta*cos)         ([src, dst] layout)
        numT = h_nm^T W, den = 1^T W      (PSUM-accumulated matmuls)
        out  = num * (den>0)/max(den,eps) (column scaling via rank-1 matmul)
  * 32 query graphs (16 nodes each) are packed into one block-diagonal
    512-node graph and use the same code path (c=0).
  * The MLP second half (broadcast query features) collapses to the rank-1
    term mask (x) (B1^T u_g), never materialized per node.
All heavy matmuls run in bf16 with fp32 PSUM accumulation.
"""

import threading
from contextlib import ExitStack

import numpy as np
import ml_dtypes

import concourse.bass as bass
import concourse.mybir as mybir
import concourse.tile as tile
from concourse import bacc
from concourse.bass import ds, ts
from concourse.bass_utils import run_bass_kernel_spmd
from concourse.masks import make_identity

BF16 = mybir.dt.bfloat16
F32 = mybir.dt.float32
AF = mybir.ActivationFunctionType
ALU = mybir.AluOpType

# problem constants
B = 256
NPG = 512
NQPG = 16
IN, H, L, OUT = 64, 128, 2, 1
NCORES = 8
GPC = B // NCORES          # graphs per core (32)
N = NPG                    # dense block size for data graphs
NCH = N // 128             # 4 chunks of 128 src nodes


NQ_FIX = 512               # padded query block (32 graphs x 16 nodes)
G32 = NQ_FIX // NQPG       # 32 query slots


def build_program(gpc=GPC):
    """Build the per-core Bass/Tile program (identical on all 8 cores)."""
    n_nodes = gpc * NPG
    nq_blk = NQ_FIX

    nc = bacc.Bacc("TRN2", target_bir_lowering=False, debug=False,
                   num_devices=NCORES)

    io = {}
    io["xt"] = nc.dram_tensor("xt", [IN, n_nodes], BF16,
                              kind="ExternalInput").ap()
    io["xqt"] = nc.dram_tensor("xqt", [IN, NQ_FIX], BF16,
                               kind="ExternalInput").ap()
    io["ct"] = nc.dram_tensor("ct", [gpc + 1, NCH, 128, N], BF16,
                              kind="ExternalInput").ap()
    for nm, shp, dt in [
        ("wg", [IN, H], BF16), ("wq", [IN, H], BF16),
        ("bgc", [H, 1], F32), ("bqc", [H, 1], F32),
        ("betg", [L, H, 1], F32), ("betq", [L, H, 1], F32),
        ("a1", [L, H, H], BF16), ("b1t", [L, H, H], BF16),
        ("w2", [L, H, H], BF16),
        ("b1c", [L, H, 1], F32), ("b2c", [L, H, 1], F32),
        ("wp1", [H, H], BF16), ("wp2", [H, 1], BF16),
        ("bp1c", [H, 1], F32), ("bp2c", [1, 1], F32),
    ]:
        io[nm] = nc.dram_tensor(nm, shp, dt, kind="ExternalInput").ap()
    io["y"] = nc.dram_tensor("y", [1, gpc], F32, kind="ExternalOutput").ap()

    with tile.TileContext(nc) as tc:
        _emit(tc, nc, gpc, io)
    nc.compile()
    return nc


def _emit(tc, nc, gpc, io):
    n_nodes = gpc * NPG
    nq_blk = NQ_FIX
    nqch = nq_blk // 128

    ctx = ExitStack()
    with ctx:
        pconst = ctx.enter_context(tc.tile_pool(name="pconst", bufs=1))
        pstate = ctx.enter_context(tc.tile_pool(name="pstate", bufs=1))
        pct = ctx.enter_context(tc.tile_pool(name="pct", bufs=2))
        pwork = ctx.enter_context(tc.tile_pool(name="pwork", bufs=3))
        prow = ctx.enter_context(tc.tile_pool(name="prow", bufs=4))
        ps_cos = ctx.enter_context(
            tc.tile_pool(name="ps_cos", bufs=2, space="PSUM"))
        ps_big = ctx.enter_context(
            tc.tile_pool(name="ps_big", bufs=2, space="PSUM"))
        ps_acc = ctx.enter_context(
            tc.tile_pool(name="ps_acc", bufs=2, space="PSUM"))
        ps_row = ctx.enter_context(
            tc.tile_pool(name="ps_row", bufs=1, space="PSUM"))

        def const(name, shape, dtype):
            return pconst.tile(shape, dtype, name=name, tag=name)

        # ---- constants ----
        ident_f = const("ident_f", [128, 128], F32)
        make_identity(nc, ident_f[:])
        ones_col_bf = const("ones_col_bf", [128, 1], BF16)
        nc.vector.memset(ones_col_bf[:], 1.0)
        ones_row_bf = const("ones_row_bf", [1, 128], BF16)
        nc.vector.memset(ones_row_bf[:], 1.0)
        qeps = const("qeps", [1, 1], F32)
        nc.vector.memset(qeps[:], 1e-24)
        ones_512f = const("ones_512f", [1, N], F32)
        nc.vector.memset(ones_512f[:], 1.0)

        # ---- load weights into SBUF ----
        def load(name, ap_dram, shape, dtype):
            t = const(name, shape, dtype)
            nc.sync.dma_start(t[:], ap_dram)
            return t

        wg_s = load("wg_s", io["wg"][:], [IN, H], BF16)
        wq_s = load("wq_s", io["wq"][:], [IN, H], BF16)
        bgc_s = load("bgc_s", io["bgc"][:], [H, 1], F32)
        bqc_s = load("bqc_s", io["bqc"][:], [H, 1], F32)
        betg_s = [load(f"betg{l}", io["betg"][l], [H, 1], F32)
                  for l in range(L)]
        betq_s = [load(f"betq{l}", io["betq"][l], [H, 1], F32)
                  for l in range(L)]
        a1_s = [load(f"a1_{l}", io["a1"][l], [H, H], BF16) for l in range(L)]
        b1t_s = [load(f"b1t_{l}", io["b1t"][l], [H, H], BF16)
                 for l in range(L)]
        w2_s = [load(f"w2_{l}", io["w2"][l], [H, H], BF16) for l in range(L)]
        b1c_s = [load(f"b1c_{l}", io["b1c"][l], [H, 1], F32)
                 for l in range(L)]
        b2c_s = [load(f"b2c_{l}", io["b2c"][l], [H, 1], F32)
                 for l in range(L)]
        wp1_s = load("wp1_s", io["wp1"][:], [H, H], BF16)
        wp2_s = load("wp2_s", io["wp2"][:], [H, 1], BF16)
        bp1c_s = load("bp1c_s", io["bp1c"][:], [H, 1], F32)
        bp2c_s = load("bp2c_s", io["bp2c"][:], [1, 1], F32)

        xqt_s = load("xqt_s", io["xqt"][:], [IN, nq_blk], BF16)

        # query-block count matrix: resident for the whole kernel
        ctq_tile = const("ctq_tile", [128, NCH, N], BF16)
        nc.sync.dma_start(ctq_tile[:],
                          io["ct"][gpc].rearrange("c p f -> p c f"))

        # ---- persistent state ----
        def state(name, shape, dtype):
            return pstate.tile(shape, dtype, name=name, tag=name)

        HGT = state("HGT", [128, n_nodes], F32)           # h_g feature-major
        HGN = state("HGN", [128, gpc * NCH * 128], BF16)  # h_g node-major bf16
        HQT = state("HQT", [128, nq_blk], F32)
        HQN = state("HQN", [128, nqch * 128], BF16)
        HGS = state("HGS", [128, gpc], F32)
        u_bf = state("u_bf", [128, G32], BF16)
        c_row = state("c_row", [1, G32], F32)
        ce_row = state("ce_row", [1, G32], F32)
        vrow = state("vrow", [1, G32 * 128], BF16)

        def hgn_sl(g, sc):
            return HGN[:, ds((g * NCH + sc) * 128, 128)]

        def make_node_major(srcT, dst_ap, k):
            """srcT [128, k*128] f32 SBUF -> transpose -> dst bf16
            [128, k*128] (node-major)."""
            tpb = ps_big.tile([128, k * 128], F32, name="tpb", tag="big")
            for i in range(k):
                nc.tensor.transpose(tpb[:, ts(i, 128)], srcT[:, ts(i, 128)],
                                    ident_f[:])
            nc.vector.tensor_copy(dst_ap, tpb[:])

        # ---- initial projections ----
        for g in range(gpc):
            xg_t = pwork.tile([IN, N], BF16, name="xg_t", tag="xg")
            nc.sync.dma_start(xg_t[:], io["xt"][:, ts(g, N)])
            xg_ps = ps_big.tile([128, N], F32, name="xg_ps", tag="big")
            nc.tensor.matmul(xg_ps[:], wg_s[:], xg_t[:],
                             start=True, stop=True)
            nc.scalar.activation(HGT[:, ts(g, N)], xg_ps[:], AF.Identity,
                                 bias=bgc_s[:])
            make_node_major(HGT[:, ts(g, N)],
                            HGN[:, ds(g * NCH * 128, NCH * 128)], NCH)

        xq_ps = ps_big.tile([128, nq_blk], F32, name="xq_ps", tag="big")
        nc.tensor.matmul(xq_ps[:], wq_s[:], xqt_s[:], start=True, stop=True)
        nc.scalar.activation(HQT[:], xq_ps[:], AF.Identity, bias=bqc_s[:])
        make_node_major(HQT[:], HQN[:], nqch)

        def agnn(hT, h_nm_slices, ct_all_ap, beta_col, c_ap, nch, nn):
            """Dense AGNN block; returns (num_ps, dm, mask_bf)."""
            sq = pwork.tile([128, nn], BF16, name="sq", tag="sq")
            nc.gpsimd.tensor_mul(sq[:], hT, hT)
            nsq_ps = ps_row.tile([1, nn], F32, name="nsq_ps", tag="row")
            nc.tensor.matmul(nsq_ps[:], ones_col_bf[:], sq[:],
                             start=True, stop=True)
            ln_row = prow.tile([1, nn], F32, name="ln_row", tag="frow")
            bias = qeps[:] if c_ap is None else c_ap
            nc.scalar.activation(ln_row[:], nsq_ps[:], AF.Ln, bias=bias)
            ir_row = prow.tile([1, nn], BF16, name="ir_row", tag="brow")
            nc.scalar.activation(ir_row[:], ln_row[:], AF.Exp, scale=-0.5)
            cir_row = None
            if c_ap is not None:
                cir_row = prow.tile([1, nn], BF16, name="cir_row", tag="brow")
                nc.vector.tensor_scalar(cir_row[:], ir_row[:], c_ap, None,
                                        op0=ALU.mult)
            irb = pwork.tile([128, nn], BF16, name="irb", tag="irb")
            nc.gpsimd.partition_broadcast(irb[:], ir_row[:])
            hn = pwork.tile([128, nn], BF16, name="hn", tag="hn")
            nc.vector.tensor_tensor(hn[:], hT, irb[:], op=ALU.mult)

            num_ps = ps_acc.tile([128, nn], F32, name="num_ps", tag="acc")
            den_ps = ps_row.tile([1, nn], F32, name="den_ps", tag="row")
            for sc in range(nch):
                cos_ps = ps_cos.tile([128, nn], F32, name="cos_ps",
                                     tag="cos")
                nc.tensor.matmul(cos_ps[:], hn[:, ts(sc, 128)],
                                 hn[:], start=True,
                                 stop=(cir_row is None))
                if cir_row is not None:
                    nc.tensor.matmul(cos_ps[:],
                                     cir_row[:, ts(sc, 128)], ir_row[:],
                                     start=False, stop=True)
                ee = pwork.tile([128, nn], BF16, name="ee", tag="ee")
                nc.scalar.activation(ee[:], cos_ps[:], AF.Exp, scale=beta_col)
                wt = pwork.tile([128, nn], BF16, name="wt", tag="wt")
                nc.vector.tensor_tensor(wt[:], ee[:], ct_all_ap[:, sc, :],
                                        op=ALU.mult)
                nc.tensor.matmul(num_ps[:], h_nm_slices[sc], wt[:],
                                 start=(sc == 0), stop=(sc == nch - 1))
                nc.tensor.matmul(den_ps[:], ones_col_bf[:], wt[:],
                                 start=(sc == 0), stop=(sc == nch - 1))

            mbar = prow.tile([1, nn], F32, name="mbar", tag="frow")
            nc.vector.tensor_scalar(mbar[:], den_ps[:], 0.0, None,
                                    op0=ALU.is_le)
            t_row = prow.tile([1, nn], F32, name="t_row", tag="frow")
            nc.vector.tensor_tensor(t_row[:], mbar[:], den_ps[:], op=ALU.add)
            dmask_f = prow.tile([1, nn], F32, name="dmask_f", tag="frow")
            nc.vector.reciprocal(dmask_f[:], t_row[:])
            mask_bf = prow.tile([1, nn], BF16, name="mask_bf", tag="brow")
            nc.gpsimd.tensor_scalar(mask_bf[:], mbar[:], 0.0, None,
                                    op0=ALU.is_equal)
            dm = pwork.tile([128, nn], F32, name="dm", tag="dm")
            nc.gpsimd.partition_broadcast(dm[:], dmask_f[:])
            return num_ps, dm, mask_bf

        for l in range(L):
            # ---- query AGNN on the packed block-diagonal graph ----
            num_ps, dm, _ = agnn(
                HQT[:], [HQN[:, ts(sc, 128)] for sc in range(nqch)],
                ctq_tile[:], betq_s[l][:], None, nqch, nq_blk)
            nc.vector.tensor_tensor(HQT[:], num_ps[:], dm[:], op=ALU.mult)
            make_node_major(HQT[:], HQN[:], nqch)

            # ---- per-graph query aggregates: u, c = |u|^2, v = B1^T u ----
            u_f = pwork.tile([128, G32], F32, name="u_f", tag="uf")
            nc.vector.tensor_reduce(
                u_f[:], HQT[:].rearrange("p (g k) -> p g k", k=NQPG),
                axis=mybir.AxisListType.X, op=ALU.add)
            nc.vector.tensor_copy(u_bf[:], u_f[:])
            squ = pwork.tile([128, G32], BF16, name="squ", tag="uf")
            nc.gpsimd.tensor_mul(squ[:], u_f[:], u_f[:])
            c_ps = ps_row.tile([1, G32], F32, name="c_ps", tag="row")
            nc.tensor.matmul(c_ps[:], ones_col_bf[:], squ[:],
                             start=True, stop=True)
            nc.vector.tensor_copy(c_row[:], c_ps[:])
            nc.vector.tensor_scalar(ce_row[:], c_ps[:], 1e-24, None,
                                    op0=ALU.add)
            # v = B1^T u for all graphs at once; flatten [G32,128] rows to
            # partition-0 [1, G32*128] via one SBUF->SBUF DMA
            v_ps = ps_big.tile([128, G32], F32, name="v_ps", tag="big")
            nc.tensor.matmul(v_ps[:], b1t_s[l][:], u_bf[:],
                             start=True, stop=True)
            v_sb = pwork.tile([128, G32], F32, name="v_sb", tag="uf")
            nc.vector.tensor_copy(v_sb[:], v_ps[:])
            vt_ps = ps_big.tile([G32, 128], F32, name="vt_ps", tag="big")
            nc.tensor.transpose(vt_ps[:], v_sb[:], ident_f[:])
            vt32 = pwork.tile([G32, 128], BF16, name="vt32", tag="vt32")
            nc.vector.tensor_copy(vt32[:], vt_ps[:])
            nc.sync.dma_start(vrow[:], vt32[:])

            # ---- data graphs, processed in pairs: elementwise/row ops run
            # 1024-wide across both graphs; matmuls/MLP remain per-graph ----
            for p in range(gpc // 2):
                gA = 2 * p
                ctg2 = pct.tile([128, 2, NCH, N], BF16, name="ctg2", tag="ct")
                nc.sync.dma_start(
                    ctg2[:],
                    io["ct"][ds(gA, 2)].rearrange("g c p f -> p g c f"))

                hTp = HGT[:, ds(gA * N, 2 * N)]
                sqp = pwork.tile([128, 2 * N], BF16, name="sqp", tag="sq")
                nc.gpsimd.tensor_mul(sqp[:], hTp, hTp)
                nsq_ps = ps_row.tile([1, 2 * N], F32, name="nsq_ps",
                                     tag="row")
                for gi in range(2):
                    nc.tensor.matmul(nsq_ps[0:1, ds(gi * N, N)],
                                     ones_col_bf[:],
                                     sqp[:, ds(gi * N, N)],
                                     start=True, stop=False)
                    # per-graph c via a K=1,M=1 rank-0 matmul
                    nc.tensor.matmul(nsq_ps[0:1, ds(gi * N, N)],
                                     c_row[0:1, ds(gA + gi, 1)],
                                     ones_512f[:],
                                     start=False, stop=True)
                lnp = prow.tile([1, 2 * N], F32, name="lnp", tag="frow")
                nc.scalar.activation(lnp[:], nsq_ps[:], AF.Ln, bias=qeps[:])
                irp = prow.tile([1, 2 * N], BF16, name="irp", tag="brow")
                nc.scalar.activation(irp[:], lnp[:], AF.Exp, scale=-0.5)
                cirp = prow.tile([1, 2 * N], BF16, name="cirp", tag="brow")
                for gi in range(2):
                    nc.vector.tensor_scalar(cirp[0:1, ds(gi * N, N)],
                                            irp[0:1, ds(gi * N, N)],
                                            c_row[0:1, ds(gA + gi, 1)], None,
                                            op0=ALU.mult)
                irbp = pwork.tile([128, 2 * N], BF16, name="irbp", tag="irb")
                nc.gpsimd.partition_broadcast(irbp[:], irp[:])
                hnp = pwork.tile([128, 2 * N], BF16, name="hnp", tag="hn")
                nc.vector.tensor_tensor(hnp[:], hTp, irbp[:], op=ALU.mult)

                den_ps = ps_row.tile([1, 2 * N], F32, name="den_ps",
                                     tag="row")
                num_pss = []
                for gi in range(2):
                    num_ps = ps_acc.tile([128, N], F32, name="num_ps",
                                         tag="acc")
                    num_pss.append(num_ps)
                    for sc in range(NCH):
                        cos_ps = ps_cos.tile([128, N], F32, name="cos_ps",
                                             tag="cos")
                        nc.tensor.matmul(
                            cos_ps[:],
                            hnp[:, ds(gi * N + sc * 128, 128)],
                            hnp[:, ds(gi * N, N)],
                            start=True, stop=False)
                        nc.tensor.matmul(
                            cos_ps[:],
                            cirp[0:1, ds(gi * N + sc * 128, 128)],
                            irp[0:1, ds(gi * N, N)],
                            start=False, stop=True)
                        ee = pwork.tile([128, N], BF16, name="ee", tag="ee")
                        nc.scalar.activation(ee[:], cos_ps[:], AF.Exp,
                                             scale=betg_s[l][:])
                        wt = pwork.tile([128, N], BF16, name="wt", tag="wt")
                        nc.vector.tensor_tensor(
                            wt[:], ee[:], ctg2[:, gi, sc, :], op=ALU.mult)
                        nc.tensor.matmul(num_ps[:], hgn_sl(gA + gi, sc),
                                         wt[:], start=(sc == 0),
                                         stop=(sc == NCH - 1))
                        nc.tensor.matmul(den_ps[0:1, ds(gi * N, N)],
                                         ones_col_bf[:], wt[:],
                                         start=(sc == 0),
                                         stop=(sc == NCH - 1))

                mbar = prow.tile([1, 2 * N], F32, name="mbar", tag="frow")
                nc.vector.tensor_scalar(mbar[:], den_ps[:], 0.0, None,
                                        op0=ALU.is_le)
                t_row = prow.tile([1, 2 * N], F32, name="t_row", tag="frow")
                nc.vector.tensor_tensor(t_row[:], mbar[:], den_ps[:],
                                        op=ALU.add)
                dmask_f = prow.tile([1, 2 * N], F32, name="dmask_f",
                                    tag="frow")
                nc.vector.reciprocal(dmask_f[:], t_row[:])
                mask_bf = prow.tile([1, 2 * N], BF16, name="mask_bf",
                                    tag="brow")
                nc.gpsimd.tensor_scalar(mask_bf[:], mbar[:], 0.0, None,
                                        op0=ALU.is_equal)
                dmp = pwork.tile([128, 2 * N], F32, name="dmp", tag="dm")
                nc.gpsimd.partition_broadcast(dmp[:], dmask_f[:])

                for gi in range(2):
                    g = gA + gi
                    s1 = pwork.tile([128, N], BF16, name="s1", tag="s1")
                    nc.vector.tensor_tensor(s1[:], num_pss[gi][:],
                                            dmp[:, ds(gi * N, N)],
                                            op=ALU.mult)
                    z_ps = ps_big.tile([128, N], F32, name="z_ps", tag="big")
                    nc.tensor.matmul(z_ps[:], a1_s[l][:], s1[:],
                                     start=True, stop=False)
                    nc.tensor.matmul(z_ps[:], vrow[0:1, ts(g, 128)],
                                     mask_bf[0:1, ds(gi * N, N)],
                                     start=False, stop=True)
                    rz = pwork.tile([128, N], BF16, name="rz", tag="s1")
                    nc.scalar.activation(rz[:], z_ps[:], AF.Relu,
                                         bias=b1c_s[l][:])
                    h2_ps = ps_acc.tile([128, N], F32, name="h2_ps",
                                        tag="acc")
                    nc.tensor.matmul(h2_ps[:], w2_s[l][:], rz[:],
                                     start=True, stop=True)
                    nc.scalar.activation(HGT[:, ts(g, N)], h2_ps[:],
                                         AF.Identity, bias=b2c_s[l][:])
                    make_node_major(HGT[:, ts(g, N)],
                                    HGN[:, ds(g * NCH * 128, NCH * 128)],
                                    NCH)
                    if l == L - 1:
                        nc.vector.tensor_reduce(
                            HGS[:, ds(g, 1)], HGT[:, ts(g, N)],
                            axis=mybir.AxisListType.X, op=ALU.add)

        # ---- final predictor ----
        hgs_bf = pwork.tile([128, gpc], BF16, name="hgs_bf", tag="uf")
        nc.vector.tensor_copy(hgs_bf[:], HGS[:])
        z1_ps = ps_big.tile([128, gpc], F32, name="z1_ps", tag="big")
        nc.tensor.matmul(z1_ps[:], wp1_s[:], hgs_bf[:], start=True, stop=True)
        r1 = pwork.tile([128, gpc], BF16, name="r1", tag="uf")
        nc.scalar.activation(r1[:], z1_ps[:], AF.Relu, bias=bp1c_s[:])
        y_ps = ps_row.tile([1, gpc], F32, name="y_ps", tag="row")
        nc.tensor.matmul(y_ps[:], wp2_s[:], r1[:], start=True, stop=True)
        y_sb = prow.tile([1, gpc], F32, name="y_sb", tag="frow")
        nc.scalar.activation(y_sb[:], y_ps[:], AF.Identity, bias=bp2c_s[:])
        nc.sync.dma_start(io["y"][:], y_sb[:])


def _build_ct_np(src, dst, npb, nblocks):
    blk = src // npb
    s = src - blk * npb
    d = dst - blk * npb
    flat = blk * (npb * npb) + s * npb + d
    cnt = np.bincount(flat, minlength=nblocks * npb * npb)
    return cnt.reshape(nblocks, npb, npb)


_PROG_CACHE = {}
_PROG_LOCK = threading.Lock()


def _get_program(gpc=GPC):
    with _PROG_LOCK:
        if gpc not in _PROG_CACHE:
            _PROG_CACHE[gpc] = build_program(gpc)
        return _PROG_CACHE[gpc]


def _make_in_maps(inputs, gpc=GPC, ncores=NCORES):
    bf = ml_dtypes.bfloat16
    X = np.asarray(inputs["X"], np.float32)
    X_q = np.asarray(inputs["X_q"], np.float32)
    g_src = np.asarray(inputs["g_src"], np.int64)
    g_dst = np.asarray(inputs["g_dst"], np.int64)
    q_src = np.asarray(inputs["q_src"], np.int64)
    q_dst = np.asarray(inputs["q_dst"], np.int64)

    W1r = np.asarray(inputs["W1r"], np.float32)
    shared = {
        "wg": np.asarray(inputs["Wg"], np.float32).astype(bf),
        "wq": np.asarray(inputs["Wq"], np.float32).astype(bf),
        "bgc": np.asarray(inputs["bg"], np.float32).reshape(H, 1).copy(),
        "bqc": np.asarray(inputs["bq"], np.float32).reshape(H, 1).copy(),
        "betg": np.tile(
            np.asarray(inputs["betas_g"], np.float32).reshape(L, 1, 1),
            (1, H, 1)),
        "betq": np.tile(
            np.asarray(inputs["betas_q"], np.float32).reshape(L, 1, 1),
            (1, H, 1)),
        "a1": np.ascontiguousarray(W1r[:, :H, :]).astype(bf),
        "b1t": np.ascontiguousarray(W1r[:, H:, :]).astype(bf),
        "w2": np.asarray(inputs["W2r"], np.float32).astype(bf),
        "b1c": np.asarray(inputs["b1r"], np.float32).reshape(L, H, 1).copy(),
        "b2c": np.asarray(inputs["b2r"], np.float32).reshape(L, H, 1).copy(),
        "wp1": np.asarray(inputs["Wp1"], np.float32).astype(bf),
        "wp2": np.asarray(inputs["Wp2"], np.float32).astype(bf),
        "bp1c": np.asarray(inputs["bp1"], np.float32).reshape(H, 1).copy(),
        "bp2c": np.asarray(inputs["bp2"], np.float32).reshape(1, 1).copy(),
    }

    n = gpc * NPG
    nq = gpc * NQPG
    ne = n * 8
    nqe = nq * 8
    in_maps = []
    for c in range(ncores):
        xc = X[c * n:(c + 1) * n]
        xqc = X_q[c * nq:(c + 1) * nq]
        gs = g_src[c * ne:(c + 1) * ne] - c * n
        gd = g_dst[c * ne:(c + 1) * ne] - c * n
        qs = q_src[c * nqe:(c + 1) * nqe] - c * nq
        qd = q_dst[c * nqe:(c + 1) * nqe] - c * nq

        ct_g = _build_ct_np(gs, gd, NPG, gpc)       # [gpc, 512, 512]
        ct_q = _build_ct_np(qs, qd, NQPG, gpc)      # [gpc, 16, 16]
        ctq_blk = np.zeros((512, 512), np.int64)
        for g in range(gpc):
            ctq_blk[g * NQPG:(g + 1) * NQPG,
                    g * NQPG:(g + 1) * NQPG] = ct_q[g]

        ct_all = np.concatenate([ct_g, ctq_blk[None]], 0)
        ct_all = ct_all.reshape(gpc + 1, NCH, 128, N).astype(bf)

        m = dict(shared)
        m["xt"] = np.ascontiguousarray(xc.T).astype(bf)
        xqt = np.zeros((IN, 512), np.float32)
        xqt[:, :nq] = xqc.T
        m["xqt"] = xqt.astype(bf)
        m["ct"] = ct_all
        in_maps.append(m)
    return in_maps


def run(inputs, trace=False, gpc=GPC):
    nc = _get_program(gpc)
    in_maps = _make_in_maps(inputs, gpc=gpc)
    res = run_bass_kernel_spmd(nc, in_maps, list(range(NCORES)), trace=trace)
    ys = [res.results[c]["y"].reshape(-1) for c in range(NCORES)]
    out = np.concatenate(ys).astype(np.float32).reshape(B, OUT)
    return out, res


def kernel(**inputs) -> np.ndarray:
    out, _ = run(inputs, trace=False)
    return out



# revision 38
# speedup vs baseline: 2.0779x; 2.0779x over previous
"""Trainium2 Bass kernel for nn_CascadeGNN (cascade AGNN over 256 graphs).

Graph-sharded SPMD over 8 NeuronCores, 32 graphs/core. v3 layout:
  * Dense per-graph AGNN: edges become a [512,512] count matrix Ct (host
    topology conversion). cos = hn^T hn + c * ir ir^T (rank-1 absorbs the
    per-graph broadcast query vector), W = Ct * exp(beta*cos).
  * num/den computed DST-major: num[d,f] = sum_s wt[s,d] h_nm[s,f] via
    stationary-wt matmuls; den via free-size-1 matmuls (near-free on PE)
    with eps accumulated in PSUM, then a [128,4] column reciprocal.
  * cos/ee/wt processed two src-chunks at a time ([128,1024] activations).
  * Node normalization (1/|h|) computed column-wise per graph with 1-row
    matmuls, then one batched Ln + Exp per layer (avoids activation-table
    thrash), transposed + DMA-flattened to rows for the rank-1 term and
    the partition broadcast. The +c bias is added once, batched.
  * Zero-in-degree mask rows precomputed on host from Ct (pure topology);
    AGNN zero-rows handled by den+eps (num==0 there).
  * Emission is software-pipelined: irb/hn prefetched one graph ahead;
    MLP/state-update of graph g-1 and norm columns of g-2 are emitted
    between fronts to fill engine gaps.
All heavy matmuls run in bf16 with fp32 PSUM accumulation.
"""

import threading
from contextlib import ExitStack

import numpy as np
import ml_dtypes

import concourse.bass as bass
import concourse.mybir as mybir
import concourse.tile as tile
from concourse import bacc
from concourse.bass import ds, ts
from concourse.bass_utils import run_bass_kernel_spmd
from concourse.masks import make_identity

BF16 = mybir.dt.bfloat16
F32 = mybir.dt.float32
AF = mybir.ActivationFunctionType
ALU = mybir.AluOpType

# problem constants
B = 256
NPG = 512
NQPG = 16
IN, H, L, OUT = 64, 128, 2, 1
NCORES = 8
GPC = B // NCORES          # graphs per core (32)
N = NPG                    # dense block size for data graphs
NCH = N // 128             # 4 chunks of 128 src nodes

NQ_FIX = 512               # padded query block (32 graphs x 16 nodes)
G32 = NQ_FIX // NQPG       # 32 query slots
GB = 8                     # graphs per rows block


def build_program(gpc=GPC, dbg=False):
    n_nodes = gpc * NPG

    nc = bacc.Bacc("TRN2", target_bir_lowering=False, debug=False,
                   num_devices=NCORES)

    io = {}
    io["xt"] = nc.dram_tensor("xt", [IN, n_nodes], BF16,
                              kind="ExternalInput").ap()
    io["xqt"] = nc.dram_tensor("xqt", [IN, NQ_FIX], BF16,
                               kind="ExternalInput").ap()
    io["ct"] = nc.dram_tensor("ct", [gpc + 1, NCH, 128, N], BF16,
                              kind="ExternalInput").ap()
    io["maskr"] = nc.dram_tensor("maskr", [1, gpc * N], BF16,
                                 kind="ExternalInput").ap()
    io["seg16"] = nc.dram_tensor("seg16", [128, NCH * G32], BF16,
                                 kind="ExternalInput").ap()
    for nm, shp, dt in [
        ("wg", [IN, H], BF16), ("wq", [IN, H], BF16),
        ("bgc", [H, 1], F32), ("bqc", [H, 1], F32),
        ("betg", [L, H, 1], F32), ("betq", [L, H, 1], F32),
        ("a1", [L, H, H], BF16), ("b1t", [L, H, H], BF16),
        ("w2", [L, H, H], BF16),
        ("b1c", [L, H, 1], F32), ("b2c", [L, H, 1], F32),
        ("wp1", [H, H], BF16), ("wp2", [H, 1], BF16),
        ("bp1c", [H, 1], F32), ("bp2c", [1, 1], F32),
    ]:
        io[nm] = nc.dram_tensor(nm, shp, dt, kind="ExternalInput").ap()
    io["y"] = nc.dram_tensor("y", [1, gpc], F32, kind="ExternalOutput").ap()
    if dbg:
        for nm, shp in [("d_ee", [128, N]), ("d_wt", [128, N]),
                        ("d_qrow", [1, NQ_FIX]), ("d_hnq", [128, N]),
                        ("d_dmq", [128, 8]),
                        ("d_hgt0", [128, gpc * N]), ("d_hqn0", [128, 512]),
                        ("d_crow0", [1, G32]), ("d_vrow0", [1, G32 * 128]),
                        ("d_rows0", [1, gpc * 2 * N]),
                        ("d_hgt1", [128, gpc * N]), ("d_hgs", [128, gpc])]:
            io[nm] = nc.dram_tensor(nm, shp, BF16,
                                    kind="ExternalOutput").ap()

    with tile.TileContext(nc) as tc:
        _emit(tc, nc, gpc, io, dbg=dbg)
    nc.compile()
    return nc


EMIT_LOG = []


def _mark(nc, label):
    # burn one id as a marker; instruction names are I-<id> so any
    # instruction with id >= n was emitted after this mark
    EMIT_LOG.append((label, nc.next_id()))


def _emit(tc, nc, gpc, io, dbg=False):
    nblk = gpc // GB

    ctx = ExitStack()
    with ctx:
        pconst = ctx.enter_context(tc.tile_pool(name="pconst", bufs=1))
        pstate = ctx.enter_context(tc.tile_pool(name="pstate", bufs=1))
        pct = ctx.enter_context(tc.tile_pool(name="pct", bufs=2))
        pq = ctx.enter_context(tc.tile_pool(name="pq", bufs=2))
        pwork = ctx.enter_context(tc.tile_pool(name="pwork", bufs=2))
        pw3 = ctx.enter_context(tc.tile_pool(name="pw3", bufs=4))
        pwt = ctx.enter_context(tc.tile_pool(name="pwt", bufs=2))
        prow = ctx.enter_context(tc.tile_pool(name="prow", bufs=2))
        prows = ctx.enter_context(tc.tile_pool(name="prows", bufs=2))
        ps_cos = ctx.enter_context(
            tc.tile_pool(name="ps_cos", bufs=2, space="PSUM"))
        ps_num = ctx.enter_context(
            tc.tile_pool(name="ps_num", bufs=2, space="PSUM"))
        ps_sml = ctx.enter_context(
            tc.tile_pool(name="ps_sml", bufs=2, space="PSUM"))
        ps_mlp = ctx.enter_context(
            tc.tile_pool(name="ps_mlp", bufs=2, space="PSUM"))

        def const(name, shape, dtype):
            return pconst.tile(shape, dtype, name=name, tag=name)

        # ---- constants ----
        ident_bf = const("ident_bf", [128, 128], BF16)
        make_identity(nc, ident_bf[:])
        ones_col_bf = const("ones_col_bf", [128, 1], BF16)
        nc.vector.memset(ones_col_bf[:], 1.0)
        ones_row_bf = const("ones_row_bf", [1, 128], BF16)
        nc.vector.memset(ones_row_bf[:], 1.0)
        eps_bf = const("eps_bf", [1, 1], BF16)
        nc.vector.memset(eps_bf[:], 1e-20)
        eps_col = const("eps_col", [128, 1], F32)
        nc.vector.memset(eps_col[:], 1e-24)

        # ---- load weights into SBUF ----
        _dma_rr = [nc.sync, nc.scalar, nc.gpsimd]

        def load(name, ap_dram, shape, dtype, _n=[0]):
            t = const(name, shape, dtype)
            _dma_rr[_n[0] % 3].dma_start(t[:], ap_dram)
            _n[0] += 1
            return t

        wg_s = load("wg_s", io["wg"][:], [IN, H], BF16)
        wq_s = load("wq_s", io["wq"][:], [IN, H], BF16)
        bgc_s = load("bgc_s", io["bgc"][:], [H, 1], F32)
        bqc_s = load("bqc_s", io["bqc"][:], [H, 1], F32)
        betg_s = [load(f"betg{l}", io["betg"][l], [H, 1], F32)
                  for l in range(L)]
        betq_s = [load(f"betq{l}", io["betq"][l], [H, 1], F32)
                  for l in range(L)]
        a1_s = [load(f"a1_{l}", io["a1"][l], [H, H], BF16) for l in range(L)]
        b1t_s = [load(f"b1t_{l}", io["b1t"][l], [H, H], BF16)
                 for l in range(L)]
        w2_s = [load(f"w2_{l}", io["w2"][l], [H, H], BF16) for l in range(L)]
        b1c_s = [load(f"b1c_{l}", io["b1c"][l], [H, 1], F32)
                 for l in range(L)]
        b2c_s = [load(f"b2c_{l}", io["b2c"][l], [H, 1], F32)
                 for l in range(L)]
        wp1_s = load("wp1_s", io["wp1"][:], [H, H], BF16)
        wp2_s = load("wp2_s", io["wp2"][:], [H, 1], BF16)
        bp1c_s = load("bp1c_s", io["bp1c"][:], [H, 1], F32)
        bp2c_s = load("bp2c_s", io["bp2c"][:], [1, 1], F32)

        xqt_s = load("xqt_s", io["xqt"][:], [IN, NQ_FIX], BF16)
        maskr_s = load("maskr_s", io["maskr"][:], [1, gpc * N], BF16)
        seg16_s = load("seg16_s", io["seg16"][:], [128, NCH * G32], BF16)

        # query-block count matrix: resident for the whole kernel
        ctq_tile = const("ctq_tile", [128, NCH, N], BF16)
        nc.sync.dma_start(ctq_tile[:],
                          io["ct"][gpc].rearrange("c p f -> p c f"))

        # ---- persistent state ----
        def state(name, shape, dtype):
            return pstate.tile(shape, dtype, name=name, tag=name)

        HGT = state("HGT", [128, gpc * N], BF16)     # h feature-major
        HGN = state("HGN", [128, gpc * N], BF16)     # h node-major
        HQT = state("HQT", [128, NQ_FIX], BF16)
        HQN = state("HQN", [128, NQ_FIX], BF16)
        HGS = state("HGS", [128, gpc], F32)

        def hgn_c(g, c):
            return HGN[:, ds(g * N + c * 128, 128)]

        def nm_transposes(src_fm, dst_nm, dtile, act_copy=False):
            """src_fm [128,512] bf16 SBUF -> 4 transposes -> node-major."""
            for c in range(NCH):
                nc.tensor.transpose(dtile[:, ts(c, 128)],
                                    src_fm[:, ts(c, 128)], ident_bf[:])
            if act_copy:
                nc.scalar.activation(dst_nm, dtile[:], AF.Copy)
            else:
                nc.vector.tensor_copy(dst_nm, dtile[:])

        def psml(name):
            # ps_sml slots are bank-sized; tiles share tag => same size
            return ps_sml.tile([128, 512], F32, name=name, tag="small")

        # ================= layer machinery =================

        def agnn_pairs(hn_t, ct_view, beta_col, nm_of, rows_b=None,
                       ir_off=0, cir_off=0):
            """Paired-chunk AGNN core: cos(+rank1), ee, wt, num/den.
            Returns (nd_ps [128,4,128], sml with den+eps in cols 0:4,
            dmc = 1/den)."""
            nd_ps = ps_num.tile([128, NCH, 128], F32, name="nd_ps",
                                tag="num")
            sml = psml("sml")
            _mark(nc, 'agnn')
            # wt for all 4 src chunks of this graph in one tile
            wtg = pwt.tile([128, NCH, N], BF16, name="wtg", tag="wtg")
            for sc in range(NCH):
                cos_ps = ps_cos.tile([128, N], F32, name="cos_ps",
                                     tag="cos")
                nc.tensor.matmul(cos_ps[:], hn_t[:, ts(sc, 128)],
                                 hn_t[:], start=True,
                                 stop=(rows_b is None))
                if rows_b is not None:
                    nc.tensor.matmul(
                        cos_ps[:],
                        rows_b[0:1, ds(cir_off + sc * 128, 128)],
                        rows_b[0:1, ds(ir_off, N)],
                        start=False, stop=True)
                eep = pw3.tile([128, N], BF16, name="eep", tag="ee")
                nc.scalar.activation(eep[:], cos_ps[:], AF.Exp,
                                     scale=beta_col)
                nc.vector.tensor_tensor(wtg[:, sc, :], eep[:],
                                        ct_view[:, sc, :],
                                        op=ALU.mult)
            # one open accumulation group per PSUM bank at a time:
            # dc-outer, contraction (sc) inner
            for dc in range(NCH):
                for sc in range(NCH):
                    nc.tensor.matmul(nd_ps[:, dc, :],
                                     wtg[:, sc, ts(dc, 128)],
                                     nm_of(sc),
                                     start=(sc == 0), stop=(sc == 3))
                for sc in range(NCH):
                    nc.tensor.matmul(sml[:, ds(dc, 1)],
                                     wtg[:, sc, ts(dc, 128)],
                                     ones_col_bf[:],
                                     start=(sc == 0), stop=False)
                nc.tensor.matmul(sml[:, ds(dc, 1)],
                                 ones_row_bf[:], eps_bf[:],
                                 start=False, stop=True)
            _mark(nc, 'recip')
            dmc = prow.tile([128, NCH], F32, name="dmc", tag="dmc")
            nc.vector.reciprocal(dmc[:], sml[:, 0:4])
            return nd_ps, sml, dmc

        def query_phase_gen(l, out):
            """Query AGNN layer l, staged as a generator so its serial
            chain can interleave with data-graph emission. Appends
            (c_row, vrow) to `out` when they are emitted."""
            # -- norm (columns) --
            sqf = pwork.tile([128, N], BF16, name="sqfq", tag="sqf")
            nc.vector.tensor_tensor(sqf[:], HQT[:], HQT[:], op=ALU.mult)
            smlq = psml("smlq")
            for c in range(NCH):
                nc.tensor.matmul(smlq[:, ds(4 + c, 1)], sqf[:, ts(c, 128)],
                                 ones_col_bf[:], start=True, stop=True)
            lnq = prow.tile([128, NCH], F32, name="lnq", tag="lnq")
            nc.scalar.activation(lnq[:], smlq[:, 4:8], AF.Ln,
                                 bias=eps_col[:])
            irq = prow.tile([128, NCH], BF16, name="irq", tag="irq")
            nc.scalar.activation(irq[:], lnq[:], AF.Exp, scale=-0.5)
            tqi_ps = ps_mlp.tile([NCH, 128], BF16, name="tqi_ps", tag="mlp")
            nc.tensor.transpose(tqi_ps[:], irq[:], ident_bf[:])
            tqi_sb = prow.tile([NCH, 128], BF16, name="tqi_sb", tag="tqi")
            nc.vector.tensor_copy(tqi_sb[:], tqi_ps[:])
            qrow = pq.tile([1, NQ_FIX], BF16, name="qrow", tag="qrow")
            nc.gpsimd.dma_start(qrow[:], tqi_sb[:])
            yield

            # -- agnn --
            irbq = pwork.tile([128, N], BF16, name="irbq", tag="irb")
            nc.gpsimd.partition_broadcast(irbq[:], qrow[:])
            hnq = pwork.tile([128, N], BF16, name="hnq", tag="hn")
            nc.gpsimd.tensor_mul(hnq[:], HQT[:], irbq[:])
            if dbg and l == 0:
                nc.sync.dma_start(io["d_qrow"][:], qrow[:])
                nc.sync.dma_start(io["d_hnq"][:], hnq[:])
            yield

            nd_ps, smlq2, dmq = agnn_pairs(
                hnq[:], ctq_tile[:], betq_s[l][:],
                lambda sc: HQN[:, ts(sc, 128)])
            if dbg and l == 0:
                ddm = prow.tile([128, 8], BF16, name="ddm", tag="ddm")
                nc.vector.tensor_copy(ddm[:, 0:4], dmq[:])
                nc.vector.tensor_copy(ddm[:, 4:8], smlq2[:, 0:4])
                nc.sync.dma_start(io["d_dmq"][:], ddm[:])
            # h_q (node-major) written straight back into HQN
            yield
            nc.vector.tensor_tensor(
                HQN[:].rearrange("p (c k) -> p c k", k=128), nd_ps[:],
                dmq[:].unsqueeze(-1).broadcast_to((128, NCH, 128)),
                op=ALU.mult)
            yield
            if l < L - 1:
                tq2_ps = ps_mlp.tile([128, N], BF16, name="tq2_ps",
                                     tag="mlp")
                for c in range(NCH):
                    nc.tensor.transpose(tq2_ps[:, ts(c, 128)],
                                        HQN[:, ts(c, 128)], ident_bf[:])
                nc.vector.tensor_copy(HQT[:], tq2_ps[:])
            yield

            # -- aggregates: u, c, v --
            u_ps = ps_mlp.tile([128, G32], F32, name="u_ps", tag="mlp")
            for c in range(NCH):
                nc.tensor.matmul(u_ps[:], HQN[:, ts(c, 128)],
                                 seg16_s[:, ts(c, G32)],
                                 start=(c == 0), stop=(c == NCH - 1))
            u_bf = pwork.tile([128, G32], BF16, name="u_bf", tag="uf")
            nc.vector.tensor_copy(u_bf[:], u_ps[:])
            yield
            squ = pwork.tile([128, G32], BF16, name="squ", tag="uf")
            nc.vector.tensor_tensor(squ[:], u_bf[:], u_bf[:], op=ALU.mult)
            c_ps = ps_mlp.tile([1, G32], F32, name="c_ps", tag="mlp")
            nc.tensor.matmul(c_ps[:], ones_col_bf[:], squ[:],
                             start=True, stop=True)
            c_row = pq.tile([1, G32], F32, name="c_row", tag="crow")
            nc.vector.tensor_copy(c_row[:], c_ps[:])
            yield

            v_ps = ps_mlp.tile([128, G32], F32, name="v_ps", tag="mlp")
            nc.tensor.matmul(v_ps[:], b1t_s[l][:], u_bf[:],
                             start=True, stop=True)
            v_sb = pwork.tile([128, G32], BF16, name="v_sb", tag="uf")
            nc.vector.tensor_copy(v_sb[:], v_ps[:])
            vt_ps = ps_mlp.tile([G32, 128], BF16, name="vt_ps", tag="mlp")
            nc.tensor.transpose(vt_ps[:], v_sb[:], ident_bf[:])
            vt32 = pwork.tile([G32, 128], BF16, name="vt32", tag="vt32")
            nc.vector.tensor_copy(vt32[:], vt_ps[:])
            vrow = pq.tile([1, G32 * 128], BF16, name="vrow", tag="vrow")
            nc.gpsimd.dma_start(vrow[:], vt32[:])
            out.append((c_row, vrow))

        def phase_a_sq(g, pool=False):
            """|h|^2 elementwise square for graph g."""
            sqf = pwork.tile([128, N], BF16, name="sqf", tag="sqf")
            if pool:
                nc.gpsimd.tensor_mul(sqf[:], HGT[:, ts(g, N)],
                                     HGT[:, ts(g, N)])
            else:
                nc.vector.tensor_tensor(sqf[:], HGT[:, ts(g, N)],
                                        HGT[:, ts(g, N)], op=ALU.mult)
            return sqf

        def phase_a_mm(g, sqf, nsqL, sml=None):
            """Reduce |h|^2 into nsqL[:, 4g:4g+4] via 1-row matmuls.
            (+c is added batched in norm_tail.)"""
            if sml is None:
                sml = psml("smla")
            for c in range(NCH):
                nc.tensor.matmul(sml[:, ds(4 + c, 1)], sqf[:, ts(c, 128)],
                                 ones_col_bf[:], start=True, stop=True)
            nc.vector.tensor_copy(nsqL[:, ds(g * NCH, NCH)], sml[:, 4:8])

        def norm_tail(l, c_row, nsqL):
            """Batched +c, Ln, Exp over all graphs' norm columns; transpose
            and flatten into per-block row tiles [ir | cir]."""
            # c4 broadcast: c per graph repeated over its 4 chunks
            c4row = prow.tile([1, gpc * NCH], F32, name="c4row", tag="c4r")
            c4v = c4row[:].rearrange("p (g k) -> p k g", k=NCH)
            for k in range(NCH):
                nc.vector.tensor_copy(c4v[:, k, :], c_row[:])
            c4bc = prow.tile([128, gpc * NCH], F32, name="c4bc", tag="c4b")
            nc.gpsimd.partition_broadcast(c4bc[:], c4row[:])
            nc.vector.tensor_tensor(nsqL[:], nsqL[:], c4bc[:], op=ALU.add)
            lnL = prow.tile([128, gpc * NCH], F32, name="lnL", tag="lnL")
            nc.scalar.activation(lnL[:], nsqL[:], AF.Ln, bias=eps_col[:])
            irL = prow.tile([128, gpc * NCH], BF16, name="irL", tag="irL")
            nc.scalar.activation(irL[:], lnL[:], AF.Exp, scale=-0.5)
            cirL = prow.tile([128, gpc * NCH], BF16, name="cirL", tag="cirL")
            nc.vector.tensor_tensor(cirL[:], irL[:], c4bc[:], op=ALU.mult)

            rows = []
            for b in range(nblk):
                w = GB * NCH
                t_ps = ps_mlp.tile([2 * w, 128], BF16, name="t_ps",
                                   tag="mlp")
                nc.tensor.transpose(t_ps[0:w, :], irL[:, ds(b * w, w)],
                                    ident_bf[:])
                nc.tensor.transpose(t_ps[w:2 * w, :], cirL[:, ds(b * w, w)],
                                    ident_bf[:])
                t_sb = prow.tile([2 * w, 128], BF16, name="t_sb", tag="tsb")
                nc.vector.tensor_copy(t_sb[:], t_ps[:])
                rows_b = prows.tile([1, 2 * w * 128], BF16,
                                    name=f"rows{b}", tag="rows")
                nc.gpsimd.dma_start(rows_b[:], t_sb[:])
                rows.append(rows_b)
            return rows

        def prep_hn(g, rows_b, gb):
            """irb broadcast + hn multiply for graph g (Pool, prefetched
            one iteration ahead of its front)."""
            irb = pwork.tile([128, N], BF16, name="irb", tag="irb")
            nc.gpsimd.partition_broadcast(irb[:],
                                          rows_b[0:1, ds(gb * 512, N)])
            hn = pw3.tile([128, N], BF16, name="hn", tag="hn")
            nc.gpsimd.tensor_mul(hn[:], HGT[:, ts(g, N)], irb[:])
            return hn

        def graph_front(l, g, rows_b, gb, hn):
            """Heavy AGNN part of graph g."""
            _mark(nc, f'front:{l}:{g}')
            ctg = pct.tile([128, NCH, N], BF16, name="ctg", tag="ct")
            nc.sync.dma_start(
                ctg[:], io["ct"][g].rearrange("c p f -> p c f"))

            nd_ps, sml, dmc = agnn_pairs(
                hn[:], ctg[:], betg_s[l][:],
                lambda sc: hgn_c(g, sc),
                rows_b=rows_b, ir_off=gb * 512,
                cir_off=GB * NCH * 128 + gb * 512)

            _mark(nc, 's1n')
            s1n = pwork.tile([128, N], BF16, name="s1n", tag="s1n")
            nc.vector.tensor_tensor(
                s1n[:].rearrange("p (c k) -> p c k", k=128), nd_ps[:],
                dmc[:].unsqueeze(-1).broadcast_to((128, NCH, 128)),
                op=ALU.mult)
            s1t_ps = ps_mlp.tile([128, N], BF16, name="s1t_ps", tag="mlp")
            for c in range(NCH):
                nc.tensor.transpose(s1t_ps[:, ts(c, 128)],
                                    s1n[:, ts(c, 128)], ident_bf[:])
            s1f = pw3.tile([128, N], BF16, name="s1f", tag="s1f")
            nc.vector.tensor_copy(s1f[:], s1t_ps[:])
            return s1f, sml

        def graph_back(l, g, s1f, vrow):
            """MLP + state updates for graph g."""
            _mark(nc, f'back:{l}:{g}')
            z_ps = ps_mlp.tile([128, N], F32, name="z_ps", tag="mlp")
            nc.tensor.matmul(z_ps[:], a1_s[l][:], s1f[:],
                             start=True, stop=False)
            nc.tensor.matmul(z_ps[:], vrow[0:1, ts(g, 128)],
                             maskr_s[0:1, ts(g, N)],
                             start=False, stop=True)
            rz = pwork.tile([128, N], BF16, name="rz", tag="rz")
            if g % 2 == 0:
                nc.scalar.activation(rz[:], z_ps[:], AF.Relu,
                                     bias=b1c_s[l][:])
            else:
                nc.vector.tensor_scalar(rz[:], z_ps[:], b1c_s[l][:], 0.0,
                                        op0=ALU.add, op1=ALU.max)
            h2_ps = ps_mlp.tile([128, N], F32, name="h2_ps", tag="mlp")
            nc.tensor.matmul(h2_ps[:], w2_s[l][:], rz[:],
                             start=True, stop=True)
            if l == L - 1:
                nc.scalar.activation(HGT[:, ts(g, N)], h2_ps[:],
                                     AF.Identity, bias=b2c_s[l][:],
                                     accum_out=HGS[:, ds(g, 1)])
            else:
                nc.scalar.activation(HGT[:, ts(g, N)], h2_ps[:],
                                     AF.Identity, bias=b2c_s[l][:])
            t_ps = ps_mlp.tile([128, N], BF16, name="t_ps2", tag="mlp")
            nm_transposes(HGT[:, ts(g, N)], HGN[:, ts(g, N)], t_ps)

        # ================= main flow =================
        # query projection
        xq_ps = ps_num.tile([128, NCH, 128], F32, name="xq_ps",
                            tag="num")
        xq_v = xq_ps[:].rearrange("p c k -> p (c k)")
        nc.tensor.matmul(xq_v, wq_s[:], xqt_s[:], start=True, stop=True)
        nc.scalar.activation(HQT[:], xq_v, AF.Identity, bias=bqc_s[:])
        tq_ps = ps_mlp.tile([128, NQ_FIX], BF16, name="tq_ps", tag="mlp")
        nm_transposes(HQT[:], HQN[:], tq_ps)

        # data init (batched x DMA) + layer-0 norm columns (no c needed);
        # query layer 0 stages interleave with the init stream
        nsqL0 = prow.tile([128, gpc * NCH], F32, name="nsqL0", tag="nsqL")
        XB = 2
        q0out = []
        q0gen = query_phase_gen(0, q0out)
        prev = None
        for g0 in range(0, gpc, XB):
            xb_t = pwork.tile([IN, XB * N], BF16, name="xb_t", tag="xg")
            nc.sync.dma_start(xb_t[:], io["xt"][:, ds(g0 * N, XB * N)])
            for k in range(XB):
                g = g0 + k
                xg_ps = ps_num.tile([128, NCH, 128], F32,
                                    name="xg_ps", tag="num")
                xg_v = xg_ps[:].rearrange("p c k -> p (c k)")
                nc.tensor.matmul(xg_v, wg_s[:], xb_t[:, ts(k, N)],
                                 start=True, stop=True)
                nc.scalar.activation(HGT[:, ts(g, N)], xg_v,
                                     AF.Identity, bias=bgc_s[:])
                if prev is not None:
                    gp, sqfp = prev
                    phase_a_mm(gp, sqfp, nsqL0)
                tg_ps = ps_mlp.tile([128, N], BF16, name="tg_ps", tag="mlp")
                nm_transposes(HGT[:, ts(g, N)], HGN[:, ts(g, N)], tg_ps)
                prev = (g, phase_a_sq(g))
                next(q0gen, None)
        phase_a_mm(prev[0], prev[1], nsqL0)
        for _ in q0gen:
            pass
        qt0 = q0out[0]
        if dbg:
            nc.sync.dma_start(io["d_hgt0"][:], HGT[:])
            nc.sync.dma_start(io["d_hqn0"][:], HQN[:])
            dcr = prow.tile([1, G32], BF16, name="dcr", tag="dcr")
            nc.vector.tensor_copy(dcr[:], qt0[0][:])
            nc.sync.dma_start(io["d_crow0"][:], dcr[:])
            nc.sync.dma_start(io["d_vrow0"][:], qt0[1][:])

        rows0 = norm_tail(0, qt0[0], nsqL0)
        if dbg:
            for b_ in range(nblk):
                nc.sync.dma_start(
                    io["d_rows0"][0:1, ds(b_ * 2 * GB * NCH * 128,
                                          2 * GB * NCH * 128)],
                    rows0[b_][:])
        q1out = []
        q1gen = query_phase_gen(1, q1out)    # staged into layer-0 phase B
        nsqL1 = prow.tile([128, gpc * NCH], F32, name="nsqL1", tag="nsqL")

        # layer 0 phase B (layer-1 norm columns piggybacked, staggered)
        hist = []
        hn_next = prep_hn(0, rows0[0], 0)
        for g in range(gpc):
            hn_cur = hn_next
            if hist and hist[-1][3] is None:
                gp, s1fp, smlp, _ = hist[-1]
                graph_back(0, gp, s1fp, qt0[1])
                hist[-1] = (gp, s1fp, smlp, phase_a_sq(gp))
            if len(hist) >= 2:
                g2, _, sml2, sqf2 = hist.pop(0)
                phase_a_mm(g2, sqf2, nsqL1)
            if g + 1 < gpc:
                hn_next = prep_hn(g + 1, rows0[(g + 1) // GB],
                                  (g + 1) % GB)
            s1f, sml = graph_front(0, g, rows0[g // GB], g % GB, hn_cur)
            hist.append((g, s1f, sml, None))
            next(q1gen, None)
        for i in range(len(hist)):
            g2, s1f2, sml2, sqf2 = hist[i]
            if sqf2 is None:
                graph_back(0, g2, s1f2, qt0[1])
                sqf2 = phase_a_sq(g2)
            phase_a_mm(g2, sqf2, nsqL1)

        for _ in q1gen:
            pass
        qt1 = q1out[0]
        if dbg:
            nc.sync.dma_start(io["d_hgt1"][:], HGT[:])
        rows1 = norm_tail(1, qt1[0], nsqL1)

        # layer 1 phase B
        pend = None
        hn_next = prep_hn(0, rows1[0], 0)
        for g in range(gpc):
            hn_cur = hn_next
            if pend is not None:
                graph_back(1, pend, s1f_p, qt1[1])
            if g + 1 < gpc:
                hn_next = prep_hn(g + 1, rows1[(g + 1) // GB],
                                  (g + 1) % GB)
            s1f, _ = graph_front(1, g, rows1[g // GB], g % GB, hn_cur)
            pend, s1f_p = g, s1f
        graph_back(1, pend, s1f_p, qt1[1])

        # ---- final predictor ----
        hgs_bf = pwork.tile([128, gpc], BF16, name="hgs_bf", tag="uf")
        nc.gpsimd.tensor_copy(hgs_bf[:], HGS[:])
        if dbg:
            nc.sync.dma_start(io["d_hgs"][:], hgs_bf[:])
        z1_ps = ps_mlp.tile([128, gpc], F32, name="z1_ps", tag="mlp")
        nc.tensor.matmul(z1_ps[:], wp1_s[:], hgs_bf[:], start=True,
                         stop=True)
        r1 = pwork.tile([128, gpc], BF16, name="r1", tag="uf")
        nc.scalar.activation(r1[:], z1_ps[:], AF.Relu, bias=bp1c_s[:])
        y_ps = ps_mlp.tile([1, gpc], F32, name="y_ps", tag="mlp")
        nc.tensor.matmul(y_ps[:], wp2_s[:], r1[:], start=True, stop=True)
        y_sb = prow.tile([1, gpc], F32, name="y_sb", tag="ysb")
        nc.scalar.activation(y_sb[:], y_ps[:], AF.Identity, bias=bp2c_s[:])
        nc.sync.dma_start(io["y"][:], y_sb[:])


def _build_ct_np(src, dst, npb, nblocks):
    blk = src // npb
    s = src - blk * npb
    d = dst - blk * npb
    flat = blk * (npb * npb) + s * npb + d
    cnt = np.bincount(flat, minlength=nblocks * npb * npb)
    return cnt.reshape(nblocks, npb, npb)


_PROG_CACHE = {}
_PROG_LOCK = threading.Lock()


def _get_program(gpc=GPC):
    with _PROG_LOCK:
        if gpc not in _PROG_CACHE:
            _PROG_CACHE[gpc] = build_program(gpc)
        return _PROG_CACHE[gpc]


def _make_in_maps(inputs, gpc=GPC, ncores=NCORES):
    bf = ml_dtypes.bfloat16
    X = np.asarray(inputs["X"], np.float32)
    X_q = np.asarray(inputs["X_q"], np.float32)
    g_src = np.asarray(inputs["g_src"], np.int64)
    g_dst = np.asarray(inputs["g_dst"], np.int64)
    q_src = np.asarray(inputs["q_src"], np.int64)
    q_dst = np.asarray(inputs["q_dst"], np.int64)

    W1r = np.asarray(inputs["W1r"], np.float32)
    seg16 = np.zeros((128, NCH, G32), np.float32)
    for c in range(NCH):
        for n in range(128):
            seg16[n, c, (c * 128 + n) // NQPG] = 1.0
    shared = {
        "wg": np.asarray(inputs["Wg"], np.float32).astype(bf),
        "wq": np.asarray(inputs["Wq"], np.float32).astype(bf),
        "bgc": np.asarray(inputs["bg"], np.float32).reshape(H, 1).copy(),
        "bqc": np.asarray(inputs["bq"], np.float32).reshape(H, 1).copy(),
        "betg": np.tile(
            np.asarray(inputs["betas_g"], np.float32).reshape(L, 1, 1),
            (1, H, 1)),
        "betq": np.tile(
            np.asarray(inputs["betas_q"], np.float32).reshape(L, 1, 1),
            (1, H, 1)),
        "a1": np.ascontiguousarray(W1r[:, :H, :]).astype(bf),
        "b1t": np.ascontiguousarray(W1r[:, H:, :]).astype(bf),
        "w2": np.asarray(inputs["W2r"], np.float32).astype(bf),
        "b1c": np.asarray(inputs["b1r"], np.float32).reshape(L, H, 1).copy(),
        "b2c": np.asarray(inputs["b2r"], np.float32).reshape(L, H, 1).copy(),
        "wp1": np.asarray(inputs["Wp1"], np.float32).astype(bf),
        "wp2": np.asarray(inputs["Wp2"], np.float32).astype(bf),
        "bp1c": np.asarray(inputs["bp1"], np.float32).reshape(H, 1).copy(),
        "bp2c": np.asarray(inputs["bp2"], np.float32).reshape(1, 1).copy(),
        "seg16": seg16.reshape(128, NCH * G32).astype(bf),
    }

    n = gpc * NPG
    nq = gpc * NQPG
    ne = n * 8
    nqe = nq * 8
    in_maps = []
    for c in range(ncores):
        xc = X[c * n:(c + 1) * n]
        xqc = X_q[c * nq:(c + 1) * nq]
        gs = g_src[c * ne:(c + 1) * ne] - c * n
        gd = g_dst[c * ne:(c + 1) * ne] - c * n
        qs = q_src[c * nqe:(c + 1) * nqe] - c * nq
        qd = q_dst[c * nqe:(c + 1) * nqe] - c * nq

        ct_g = _build_ct_np(gs, gd, NPG, gpc)       # [gpc, 512, 512]
        ct_q = _build_ct_np(qs, qd, NQPG, gpc)      # [gpc, 16, 16]
        ctq_blk = np.zeros((512, 512), np.int64)
        for g in range(gpc):
            ctq_blk[g * NQPG:(g + 1) * NQPG,
                    g * NQPG:(g + 1) * NQPG] = ct_q[g]

        ct_all = np.concatenate([ct_g, ctq_blk[None]], 0)
        ct_all = ct_all.reshape(gpc + 1, NCH, 128, N).astype(bf)
        maskr = (ct_g.sum(axis=1) > 0).astype(np.float32)  # [gpc, 512]

        m = dict(shared)
        m["xt"] = np.ascontiguousarray(xc.T).astype(bf)
        xqt = np.zeros((IN, 512), np.float32)
        xqt[:, :nq] = xqc.T
        m["xqt"] = xqt.astype(bf)
        m["ct"] = ct_all
        m["maskr"] = maskr.reshape(1, gpc * N).astype(bf)
        in_maps.append(m)
    return in_maps


def run(inputs, trace=False, gpc=GPC):
    nc = _get_program(gpc)
    in_maps = _make_in_maps(inputs, gpc=gpc)
    res = run_bass_kernel_spmd(nc, in_maps, list(range(NCORES)), trace=trace)
    ys = [res.results[c]["y"].reshape(-1) for c in range(NCORES)]
    out = np.concatenate(ys).astype(np.float32).reshape(B, OUT)
    return out, res


def kernel(**inputs) -> np.ndarray:
    out, _ = run(inputs, trace=False)
    return out


# revision 42
# speedup vs baseline: 2.0780x; 1.0000x over previous
"""Trainium2 Bass kernel for nn_CascadeGNN (cascade AGNN over 256 graphs).

Graph-sharded SPMD over 8 NeuronCores, 32 graphs/core. v3 layout:
  * Dense per-graph AGNN: edges become a [512,512] count matrix Ct (host
    topology conversion). cos = hn^T hn + c * ir ir^T (rank-1 absorbs the
    per-graph broadcast query vector), W = Ct * exp(beta*cos).
  * num/den computed DST-major: num[d,f] = sum_s wt[s,d] h_nm[s,f] via
    stationary-wt matmuls; den via free-size-1 matmuls (near-free on PE)
    with eps accumulated in PSUM, then a [128,4] column reciprocal.
  * cos/ee/wt processed two src-chunks at a time ([128,1024] activations).
  * Node normalization (1/|h|) computed column-wise per graph with 1-row
    matmuls, then one batched Ln + Exp per layer (avoids activation-table
    thrash), transposed + DMA-flattened to rows for the rank-1 term and
    the partition broadcast. The +c bias is added once, batched.
  * Zero-in-degree mask rows precomputed on host from Ct (pure topology);
    AGNN zero-rows handled by den+eps (num==0 there).
  * Emission is software-pipelined: irb/hn prefetched one graph ahead;
    MLP/state-update of graph g-1 and norm columns of g-2 are emitted
    between fronts to fill engine gaps.
All heavy matmuls run in bf16 with fp32 PSUM accumulation.
"""

import threading
from contextlib import ExitStack

import numpy as np
import ml_dtypes

import concourse.bass as bass
import concourse.mybir as mybir
import concourse.tile as tile
from concourse import bacc
from concourse.bass import ds, ts
from concourse.bass_utils import run_bass_kernel_spmd
from concourse.masks import make_identity

BF16 = mybir.dt.bfloat16
F32 = mybir.dt.float32
AF = mybir.ActivationFunctionType
ALU = mybir.AluOpType

# problem constants
B = 256
NPG = 512
NQPG = 16
IN, H, L, OUT = 64, 128, 2, 1
NCORES = 8
GPC = B // NCORES          # graphs per core (32)
N = NPG                    # dense block size for data graphs
NCH = N // 128             # 4 chunks of 128 src nodes

NQ_FIX = 512               # padded query block (32 graphs x 16 nodes)
G32 = NQ_FIX // NQPG       # 32 query slots
GB = 8                     # graphs per rows block


def build_program(gpc=GPC, dbg=False):
    n_nodes = gpc * NPG

    nc = bacc.Bacc("TRN2", target_bir_lowering=False, debug=False,
                   num_devices=NCORES)

    io = {}
    io["xt"] = nc.dram_tensor("xt", [IN, n_nodes], BF16,
                              kind="ExternalInput").ap()
    io["xqt"] = nc.dram_tensor("xqt", [IN, NQ_FIX], BF16,
                               kind="ExternalInput").ap()
    io["ct"] = nc.dram_tensor("ct", [gpc + 1, NCH, 128, N], BF16,
                              kind="ExternalInput").ap()
    io["maskr"] = nc.dram_tensor("maskr", [1, gpc * N], BF16,
                                 kind="ExternalInput").ap()
    io["seg16"] = nc.dram_tensor("seg16", [128, NCH * G32], BF16,
                                 kind="ExternalInput").ap()
    for nm, shp, dt in [
        ("wg", [IN, H], BF16), ("wq", [IN, H], BF16),
        ("bgc", [H, 1], F32), ("bqc", [H, 1], F32),
        ("betg", [L, H, 1], F32), ("betq", [L, H, 1], F32),
        ("a1", [L, H, H], BF16), ("b1t", [L, H, H], BF16),
        ("w2", [L, H, H], BF16),
        ("b1c", [L, H, 1], F32), ("b2c", [L, H, 1], F32),
        ("wp1", [H, H], BF16), ("wp2", [H, 1], BF16),
        ("bp1c", [H, 1], F32), ("bp2c", [1, 1], F32),
    ]:
        io[nm] = nc.dram_tensor(nm, shp, dt, kind="ExternalInput").ap()
    io["y"] = nc.dram_tensor("y", [1, gpc], F32, kind="ExternalOutput").ap()
    if dbg:
        for nm, shp in [("d_ee", [128, N]), ("d_wt", [128, N]),
                        ("d_qrow", [1, NQ_FIX]), ("d_hnq", [128, N]),
                        ("d_dmq", [128, 8]),
                        ("d_hgt0", [128, gpc * N]), ("d_hqn0", [128, 512]),
                        ("d_crow0", [1, G32]), ("d_vrow0", [1, G32 * 128]),
                        ("d_rows0", [1, gpc * 2 * N]),
                        ("d_hgt1", [128, gpc * N]), ("d_hgs", [128, gpc])]:
            io[nm] = nc.dram_tensor(nm, shp, BF16,
                                    kind="ExternalOutput").ap()

    with tile.TileContext(nc) as tc:
        _emit(tc, nc, gpc, io, dbg=dbg)
    nc.compile()
    return nc


EMIT_LOG = []


def _mark(nc, label):
    # burn one id as a marker; instruction names are I-<id> so any
    # instruction with id >= n was emitted after this mark
    EMIT_LOG.append((label, nc.next_id()))


def _emit(tc, nc, gpc, io, dbg=False):
    nblk = gpc // GB

    ctx = ExitStack()
    with ctx:
        pconst = ctx.enter_context(tc.tile_pool(name="pconst", bufs=1))
        pstate = ctx.enter_context(tc.tile_pool(name="pstate", bufs=1))
        pct = ctx.enter_context(tc.tile_pool(name="pct", bufs=2))
        pq = ctx.enter_context(tc.tile_pool(name="pq", bufs=2))
        pwork = ctx.enter_context(tc.tile_pool(name="pwork", bufs=2))
        pw3 = ctx.enter_context(tc.tile_pool(name="pw3", bufs=4))
        pwt = ctx.enter_context(tc.tile_pool(name="pwt", bufs=2))
        prow = ctx.enter_context(tc.tile_pool(name="prow", bufs=2))
        prows = ctx.enter_context(tc.tile_pool(name="prows", bufs=2))
        ps_cos = ctx.enter_context(
            tc.tile_pool(name="ps_cos", bufs=2, space="PSUM"))
        ps_num = ctx.enter_context(
            tc.tile_pool(name="ps_num", bufs=2, space="PSUM"))
        ps_sml = ctx.enter_context(
            tc.tile_pool(name="ps_sml", bufs=2, space="PSUM"))
        ps_mlp = ctx.enter_context(
            tc.tile_pool(name="ps_mlp", bufs=2, space="PSUM"))

        def const(name, shape, dtype):
            return pconst.tile(shape, dtype, name=name, tag=name)

        # ---- constants ----
        ident_bf = const("ident_bf", [128, 128], BF16)
        make_identity(nc, ident_bf[:])
        ones_col_bf = const("ones_col_bf", [128, 1], BF16)
        nc.vector.memset(ones_col_bf[:], 1.0)
        ones_row_bf = const("ones_row_bf", [1, 128], BF16)
        nc.vector.memset(ones_row_bf[:], 1.0)
        eps_bf = const("eps_bf", [1, 1], BF16)
        nc.vector.memset(eps_bf[:], 1e-20)
        eps_col = const("eps_col", [128, 1], F32)
        nc.vector.memset(eps_col[:], 1e-24)

        # ---- load weights into SBUF ----
        _dma_rr = [nc.sync, nc.scalar, nc.gpsimd]

        def load(name, ap_dram, shape, dtype, _n=[0]):
            t = const(name, shape, dtype)
            _dma_rr[_n[0] % 3].dma_start(t[:], ap_dram)
            _n[0] += 1
            return t

        wg_s = load("wg_s", io["wg"][:], [IN, H], BF16)
        wq_s = load("wq_s", io["wq"][:], [IN, H], BF16)
        bgc_s = load("bgc_s", io["bgc"][:], [H, 1], F32)
        bqc_s = load("bqc_s", io["bqc"][:], [H, 1], F32)
        betg_s = [load(f"betg{l}", io["betg"][l], [H, 1], F32)
                  for l in range(L)]
        betq_s = [load(f"betq{l}", io["betq"][l], [H, 1], F32)
                  for l in range(L)]
        a1_s = [load(f"a1_{l}", io["a1"][l], [H, H], BF16) for l in range(L)]
        b1t_s = [load(f"b1t_{l}", io["b1t"][l], [H, H], BF16)
                 for l in range(L)]
        w2_s = [load(f"w2_{l}", io["w2"][l], [H, H], BF16) for l in range(L)]
        b1c_s = [load(f"b1c_{l}", io["b1c"][l], [H, 1], F32)
                 for l in range(L)]
        b2c_s = [load(f"b2c_{l}", io["b2c"][l], [H, 1], F32)
                 for l in range(L)]
        wp1_s = load("wp1_s", io["wp1"][:], [H, H], BF16)
        wp2_s = load("wp2_s", io["wp2"][:], [H, 1], BF16)
        bp1c_s = load("bp1c_s", io["bp1c"][:], [H, 1], F32)
        bp2c_s = load("bp2c_s", io["bp2c"][:], [1, 1], F32)

        xqt_s = load("xqt_s", io["xqt"][:], [IN, NQ_FIX], BF16)
        maskr_s = load("maskr_s", io["maskr"][:], [1, gpc * N], BF16)
        seg16_s = load("seg16_s", io["seg16"][:], [128, NCH * G32], BF16)

        # query-block count matrix: resident for the whole kernel
        ctq_tile = const("ctq_tile", [128, NCH, N], BF16)
        nc.sync.dma_start(ctq_tile[:],
                          io["ct"][gpc].rearrange("c p f -> p c f"))

        # ---- persistent state ----
        def state(name, shape, dtype):
            return pstate.tile(shape, dtype, name=name, tag=name)

        HGT = state("HGT", [128, gpc * N], BF16)     # h feature-major
        HGN = state("HGN", [128, gpc * N], BF16)     # h node-major
        HQT = state("HQT", [128, NQ_FIX], BF16)
        HQN = state("HQN", [128, NQ_FIX], BF16)
        HGS = state("HGS", [128, gpc], F32)

        def hgn_c(g, c):
            return HGN[:, ds(g * N + c * 128, 128)]

        def nm_transposes(src_fm, dst_nm, dtile, act_copy=False):
            """src_fm [128,512] bf16 SBUF -> 4 transposes -> node-major."""
            for c in range(NCH):
                nc.tensor.transpose(dtile[:, ts(c, 128)],
                                    src_fm[:, ts(c, 128)], ident_bf[:])
            if act_copy:
                nc.scalar.activation(dst_nm, dtile[:], AF.Copy)
            else:
                nc.vector.tensor_copy(dst_nm, dtile[:])

        def psml(name):
            # ps_sml slots are bank-sized; tiles share tag => same size
            return ps_sml.tile([128, 512], F32, name=name, tag="small")

        # ================= layer machinery =================

        def agnn_pairs(hn_t, ct_view, beta_col, nm_of, rows_b=None,
                       ir_off=0, cir_off=0):
            """Paired-chunk AGNN core: cos(+rank1), ee, wt, num/den.
            Returns (nd_ps [128,4,128], sml with den+eps in cols 0:4,
            dmc = 1/den)."""
            nd_ps = ps_num.tile([128, NCH, 128], F32, name="nd_ps",
                                tag="num")
            sml = psml("sml")
            _mark(nc, 'agnn')
            # wt for all 4 src chunks of this graph in one tile
            wtg = pwt.tile([128, NCH, N], BF16, name="wtg", tag="wtg")
            for sc in range(NCH):
                cos_ps = ps_cos.tile([128, N], F32, name="cos_ps",
                                     tag="cos")
                nc.tensor.matmul(cos_ps[:], hn_t[:, ts(sc, 128)],
                                 hn_t[:], start=True,
                                 stop=(rows_b is None))
                if rows_b is not None:
                    nc.tensor.matmul(
                        cos_ps[:],
                        rows_b[0:1, ds(cir_off + sc * 128, 128)],
                        rows_b[0:1, ds(ir_off, N)],
                        start=False, stop=True)
                eep = pw3.tile([128, N], BF16, name="eep", tag="ee")
                nc.scalar.activation(eep[:], cos_ps[:], AF.Exp,
                                     scale=beta_col)
                nc.vector.tensor_tensor(wtg[:, sc, :], eep[:],
                                        ct_view[:, sc, :],
                                        op=ALU.mult)
            # one open accumulation group per PSUM bank at a time:
            # dc-outer, contraction (sc) inner
            for dc in range(NCH):
                for sc in range(NCH):
                    nc.tensor.matmul(nd_ps[:, dc, :],
                                     wtg[:, sc, ts(dc, 128)],
                                     nm_of(sc),
                                     start=(sc == 0), stop=(sc == 3))
                for sc in range(NCH):
                    nc.tensor.matmul(sml[:, ds(dc, 1)],
                                     wtg[:, sc, ts(dc, 128)],
                                     ones_col_bf[:],
                                     start=(sc == 0), stop=False)
                nc.tensor.matmul(sml[:, ds(dc, 1)],
                                 ones_row_bf[:], eps_bf[:],
                                 start=False, stop=True)
            _mark(nc, 'recip')
            dmc = prow.tile([128, NCH], F32, name="dmc", tag="dmc")
            nc.vector.reciprocal(dmc[:], sml[:, 0:4])
            return nd_ps, sml, dmc

        def query_phase_gen(l, out):
            """Query AGNN layer l, staged as a generator so its serial
            chain can interleave with data-graph emission. Appends
            (c_row, vrow) to `out` when they are emitted."""
            # -- norm (columns) --
            sqf = pwork.tile([128, N], BF16, name="sqfq", tag="sqf")
            nc.vector.tensor_tensor(sqf[:], HQT[:], HQT[:], op=ALU.mult)
            smlq = psml("smlq")
            for c in range(NCH):
                nc.tensor.matmul(smlq[:, ds(4 + c, 1)], sqf[:, ts(c, 128)],
                                 ones_col_bf[:], start=True, stop=True)
            lnq = prow.tile([128, NCH], F32, name="lnq", tag="lnq")
            nc.scalar.activation(lnq[:], smlq[:, 4:8], AF.Ln,
                                 bias=eps_col[:])
            irq = prow.tile([128, NCH], BF16, name="irq", tag="irq")
            nc.scalar.activation(irq[:], lnq[:], AF.Exp, scale=-0.5)
            tqi_ps = ps_mlp.tile([NCH, 128], BF16, name="tqi_ps", tag="mlp")
            nc.tensor.transpose(tqi_ps[:], irq[:], ident_bf[:])
            tqi_sb = prow.tile([NCH, 128], BF16, name="tqi_sb", tag="tqi")
            nc.vector.tensor_copy(tqi_sb[:], tqi_ps[:])
            qrow = pq.tile([1, NQ_FIX], BF16, name="qrow", tag="qrow")
            nc.gpsimd.dma_start(qrow[:], tqi_sb[:])
            yield

            # -- agnn --
            irbq = pwork.tile([128, N], BF16, name="irbq", tag="irb")
            nc.gpsimd.partition_broadcast(irbq[:], qrow[:])
            hnq = pwork.tile([128, N], BF16, name="hnq", tag="hn")
            nc.gpsimd.tensor_mul(hnq[:], HQT[:], irbq[:])
            if dbg and l == 0:
                nc.sync.dma_start(io["d_qrow"][:], qrow[:])
                nc.sync.dma_start(io["d_hnq"][:], hnq[:])
            yield

            nd_ps, smlq2, dmq = agnn_pairs(
                hnq[:], ctq_tile[:], betq_s[l][:],
                lambda sc: HQN[:, ts(sc, 128)])
            if dbg and l == 0:
                ddm = prow.tile([128, 8], BF16, name="ddm", tag="ddm")
                nc.vector.tensor_copy(ddm[:, 0:4], dmq[:])
                nc.vector.tensor_copy(ddm[:, 4:8], smlq2[:, 0:4])
                nc.sync.dma_start(io["d_dmq"][:], ddm[:])
            # h_q (node-major) written straight back into HQN
            yield
            nc.vector.tensor_tensor(
                HQN[:].rearrange("p (c k) -> p c k", k=128), nd_ps[:],
                dmq[:].unsqueeze(-1).broadcast_to((128, NCH, 128)),
                op=ALU.mult)
            yield
            if l < L - 1:
                tq2_ps = ps_mlp.tile([128, N], BF16, name="tq2_ps",
                                     tag="mlp")
                for c in range(NCH):
                    nc.tensor.transpose(tq2_ps[:, ts(c, 128)],
                                        HQN[:, ts(c, 128)], ident_bf[:])
                nc.vector.tensor_copy(HQT[:], tq2_ps[:])
            yield

            # -- aggregates: u, c, v --
            u_ps = ps_mlp.tile([128, G32], F32, name="u_ps", tag="mlp")
            for c in range(NCH):
                nc.tensor.matmul(u_ps[:], HQN[:, ts(c, 128)],
                                 seg16_s[:, ts(c, G32)],
                                 start=(c == 0), stop=(c == NCH - 1))
            u_bf = pwork.tile([128, G32], BF16, name="u_bf", tag="uf")
            nc.vector.tensor_copy(u_bf[:], u_ps[:])
            yield
            squ = pwork.tile([128, G32], BF16, name="squ", tag="uf")
            nc.vector.tensor_tensor(squ[:], u_bf[:], u_bf[:], op=ALU.mult)
            c_ps = ps_mlp.tile([1, G32], F32, name="c_ps", tag="mlp")
            nc.tensor.matmul(c_ps[:], ones_col_bf[:], squ[:],
                             start=True, stop=True)
            c_row = pq.tile([1, G32], F32, name="c_row", tag="crow")
            nc.vector.tensor_copy(c_row[:], c_ps[:])
            yield

            v_ps = ps_mlp.tile([128, G32], F32, name="v_ps", tag="mlp")
            nc.tensor.matmul(v_ps[:], b1t_s[l][:], u_bf[:],
                             start=True, stop=True)
            v_sb = pwork.tile([128, G32], BF16, name="v_sb", tag="uf")
            nc.vector.tensor_copy(v_sb[:], v_ps[:])
            vt_ps = ps_mlp.tile([G32, 128], BF16, name="vt_ps", tag="mlp")
            nc.tensor.transpose(vt_ps[:], v_sb[:], ident_bf[:])
            vt32 = pwork.tile([G32, 128], BF16, name="vt32", tag="vt32")
            nc.vector.tensor_copy(vt32[:], vt_ps[:])
            vrow = pq.tile([1, G32 * 128], BF16, name="vrow", tag="vrow")
            nc.gpsimd.dma_start(vrow[:], vt32[:])
            out.append((c_row, vrow))

        def phase_a_sq(g, pool=False):
            """|h|^2 elementwise square for graph g."""
            sqf = pwork.tile([128, N], BF16, name="sqf", tag="sqf")
            if pool:
                nc.gpsimd.tensor_mul(sqf[:], HGT[:, ts(g, N)],
                                     HGT[:, ts(g, N)])
            else:
                nc.vector.tensor_tensor(sqf[:], HGT[:, ts(g, N)],
                                        HGT[:, ts(g, N)], op=ALU.mult)
            return sqf

        def phase_a_mm(g, sqf, nsqL, sml=None):
            """Reduce |h|^2 into nsqL[:, 4g:4g+4] via 1-row matmuls.
            (+c is added batched in norm_tail.)"""
            if sml is None:
                sml = psml("smla")
            for c in range(NCH):
                nc.tensor.matmul(sml[:, ds(4 + c, 1)], sqf[:, ts(c, 128)],
                                 ones_col_bf[:], start=True, stop=True)
            nc.vector.tensor_copy(nsqL[:, ds(g * NCH, NCH)], sml[:, 4:8])

        def norm_tail(l, c_row, nsqL):
            """Batched +c, Ln, Exp over all graphs' norm columns; transpose
            and flatten into per-block row tiles [ir | cir]."""
            # c4 broadcast: c per graph repeated over its 4 chunks
            c4row = prow.tile([1, gpc * NCH], F32, name="c4row", tag="c4r")
            c4v = c4row[:].rearrange("p (g k) -> p k g", k=NCH)
            for k in range(NCH):
                nc.vector.tensor_copy(c4v[:, k, :], c_row[:])
            c4bc = prow.tile([128, gpc * NCH], F32, name="c4bc", tag="c4b")
            nc.gpsimd.partition_broadcast(c4bc[:], c4row[:])
            nc.vector.tensor_tensor(nsqL[:], nsqL[:], c4bc[:], op=ALU.add)
            lnL = prow.tile([128, gpc * NCH], F32, name="lnL", tag="lnL")
            nc.scalar.activation(lnL[:], nsqL[:], AF.Ln, bias=eps_col[:])
            irL = prow.tile([128, gpc * NCH], BF16, name="irL", tag="irL")
            nc.scalar.activation(irL[:], lnL[:], AF.Exp, scale=-0.5)
            cirL = prow.tile([128, gpc * NCH], BF16, name="cirL", tag="cirL")
            nc.vector.tensor_tensor(cirL[:], irL[:], c4bc[:], op=ALU.mult)

            rows = []
            for b in range(nblk):
                w = GB * NCH
                t_ps = ps_mlp.tile([2 * w, 128], BF16, name="t_ps",
                                   tag="mlp")
                nc.tensor.transpose(t_ps[0:w, :], irL[:, ds(b * w, w)],
                                    ident_bf[:])
                nc.tensor.transpose(t_ps[w:2 * w, :], cirL[:, ds(b * w, w)],
                                    ident_bf[:])
                t_sb = prow.tile([2 * w, 128], BF16, name="t_sb", tag="tsb")
                nc.vector.tensor_copy(t_sb[:], t_ps[:])
                rows_b = prows.tile([1, 2 * w * 128], BF16,
                                    name=f"rows{b}", tag="rows")
                nc.gpsimd.dma_start(rows_b[:], t_sb[:])
                rows.append(rows_b)
            return rows

        def prep_hn(g, rows_b, gb):
            """irb broadcast + hn multiply for graph g (Pool, prefetched
            one iteration ahead of its front)."""
            irb = pwork.tile([128, N], BF16, name="irb", tag="irb")
            nc.gpsimd.partition_broadcast(irb[:],
                                          rows_b[0:1, ds(gb * 512, N)])
            hn = pw3.tile([128, N], BF16, name="hn", tag="hn")
            nc.vector.tensor_tensor(hn[:], HGT[:, ts(g, N)], irb[:],
                                    op=ALU.mult)
            return hn

        def graph_front(l, g, rows_b, gb, hn):
            """Heavy AGNN part of graph g."""
            _mark(nc, f'front:{l}:{g}')
            ctg = pct.tile([128, NCH, N], BF16, name="ctg", tag="ct")
            nc.sync.dma_start(
                ctg[:], io["ct"][g].rearrange("c p f -> p c f"))

            nd_ps, sml, dmc = agnn_pairs(
                hn[:], ctg[:], betg_s[l][:],
                lambda sc: hgn_c(g, sc),
                rows_b=rows_b, ir_off=gb * 512,
                cir_off=GB * NCH * 128 + gb * 512)

            _mark(nc, 's1n')
            s1n = pwork.tile([128, N], BF16, name="s1n", tag="s1n")
            nc.vector.tensor_tensor(
                s1n[:].rearrange("p (c k) -> p c k", k=128), nd_ps[:],
                dmc[:].unsqueeze(-1).broadcast_to((128, NCH, 128)),
                op=ALU.mult)
            s1t_ps = ps_mlp.tile([128, N], BF16, name="s1t_ps", tag="mlp")
            for c in range(NCH):
                nc.tensor.transpose(s1t_ps[:, ts(c, 128)],
                                    s1n[:, ts(c, 128)], ident_bf[:])
            s1f = pw3.tile([128, N], BF16, name="s1f", tag="s1f")
            nc.vector.tensor_copy(s1f[:], s1t_ps[:])
            return s1f, sml

        def graph_back(l, g, s1f, vrow):
            """MLP + state updates for graph g."""
            _mark(nc, f'back:{l}:{g}')
            z_ps = ps_mlp.tile([128, N], F32, name="z_ps", tag="mlp")
            nc.tensor.matmul(z_ps[:], a1_s[l][:], s1f[:],
                             start=True, stop=False)
            nc.tensor.matmul(z_ps[:], vrow[0:1, ts(g, 128)],
                             maskr_s[0:1, ts(g, N)],
                             start=False, stop=True)
            rz = pwork.tile([128, N], BF16, name="rz", tag="rz")
            if g % 2 == 0:
                nc.scalar.activation(rz[:], z_ps[:], AF.Relu,
                                     bias=b1c_s[l][:])
            else:
                nc.vector.tensor_scalar(rz[:], z_ps[:], b1c_s[l][:], 0.0,
                                        op0=ALU.add, op1=ALU.max)
            h2_ps = ps_mlp.tile([128, N], F32, name="h2_ps", tag="mlp")
            nc.tensor.matmul(h2_ps[:], w2_s[l][:], rz[:],
                             start=True, stop=True)
            if l == L - 1:
                nc.scalar.activation(HGT[:, ts(g, N)], h2_ps[:],
                                     AF.Identity, bias=b2c_s[l][:],
                                     accum_out=HGS[:, ds(g, 1)])
            else:
                nc.scalar.activation(HGT[:, ts(g, N)], h2_ps[:],
                                     AF.Identity, bias=b2c_s[l][:])
            t_ps = ps_mlp.tile([128, N], BF16, name="t_ps2", tag="mlp")
            nm_transposes(HGT[:, ts(g, N)], HGN[:, ts(g, N)], t_ps)

        # ================= main flow =================
        # query projection
        xq_ps = ps_num.tile([128, NCH, 128], F32, name="xq_ps",
                            tag="num")
        xq_v = xq_ps[:].rearrange("p c k -> p (c k)")
        nc.tensor.matmul(xq_v, wq_s[:], xqt_s[:], start=True, stop=True)
        nc.scalar.activation(HQT[:], xq_v, AF.Identity, bias=bqc_s[:])
        tq_ps = ps_mlp.tile([128, NQ_FIX], BF16, name="tq_ps", tag="mlp")
        nm_transposes(HQT[:], HQN[:], tq_ps)

        # data init (batched x DMA) + layer-0 norm columns (no c needed);
        # query layer 0 stages interleave with the init stream
        nsqL0 = prow.tile([128, gpc * NCH], F32, name="nsqL0", tag="nsqL")
        XB = 2
        q0out = []
        q0gen = query_phase_gen(0, q0out)
        prev = None
        for g0 in range(0, gpc, XB):
            xb_t = pwork.tile([IN, XB * N], BF16, name="xb_t", tag="xg")
            nc.sync.dma_start(xb_t[:], io["xt"][:, ds(g0 * N, XB * N)])
            for k in range(XB):
                g = g0 + k
                xg_ps = ps_num.tile([128, NCH, 128], F32,
                                    name="xg_ps", tag="num")
                xg_v = xg_ps[:].rearrange("p c k -> p (c k)")
                nc.tensor.matmul(xg_v, wg_s[:], xb_t[:, ts(k, N)],
                                 start=True, stop=True)
                nc.scalar.activation(HGT[:, ts(g, N)], xg_v,
                                     AF.Identity, bias=bgc_s[:])
                if prev is not None:
                    gp, sqfp = prev
                    phase_a_mm(gp, sqfp, nsqL0)
                tg_ps = ps_mlp.tile([128, N], BF16, name="tg_ps", tag="mlp")
                nm_transposes(HGT[:, ts(g, N)], HGN[:, ts(g, N)], tg_ps)
                prev = (g, phase_a_sq(g))
                next(q0gen, None)
        phase_a_mm(prev[0], prev[1], nsqL0)
        for _ in q0gen:
            pass
        qt0 = q0out[0]
        if dbg:
            nc.sync.dma_start(io["d_hgt0"][:], HGT[:])
            nc.sync.dma_start(io["d_hqn0"][:], HQN[:])
            dcr = prow.tile([1, G32], BF16, name="dcr", tag="dcr")
            nc.vector.tensor_copy(dcr[:], qt0[0][:])
            nc.sync.dma_start(io["d_crow0"][:], dcr[:])
            nc.sync.dma_start(io["d_vrow0"][:], qt0[1][:])

        rows0 = norm_tail(0, qt0[0], nsqL0)
        if dbg:
            for b_ in range(nblk):
                nc.sync.dma_start(
                    io["d_rows0"][0:1, ds(b_ * 2 * GB * NCH * 128,
                                          2 * GB * NCH * 128)],
                    rows0[b_][:])
        q1out = []
        q1gen = query_phase_gen(1, q1out)    # staged into layer-0 phase B
        nsqL1 = prow.tile([128, gpc * NCH], F32, name="nsqL1", tag="nsqL")

        # layer 0 phase B (layer-1 norm columns piggybacked, staggered)
        hist = []
        hn_next = prep_hn(0, rows0[0], 0)
        for g in range(gpc):
            hn_cur = hn_next
            if hist and hist[-1][3] is None:
                gp, s1fp, smlp, _ = hist[-1]
                graph_back(0, gp, s1fp, qt0[1])
                hist[-1] = (gp, s1fp, smlp, phase_a_sq(gp))
            if len(hist) >= 2:
                g2, _, sml2, sqf2 = hist.pop(0)
                phase_a_mm(g2, sqf2, nsqL1)
            if g + 1 < gpc:
                hn_next = prep_hn(g + 1, rows0[(g + 1) // GB],
                                  (g + 1) % GB)
            s1f, sml = graph_front(0, g, rows0[g // GB], g % GB, hn_cur)
            hist.append((g, s1f, sml, None))
            next(q1gen, None)
        for i in range(len(hist)):
            g2, s1f2, sml2, sqf2 = hist[i]
            if sqf2 is None:
                graph_back(0, g2, s1f2, qt0[1])
                sqf2 = phase_a_sq(g2)
            phase_a_mm(g2, sqf2, nsqL1)

        for _ in q1gen:
            pass
        qt1 = q1out[0]
        if dbg:
            nc.sync.dma_start(io["d_hgt1"][:], HGT[:])
        rows1 = norm_tail(1, qt1[0], nsqL1)

        # layer 1 phase B
        pend = None
        hn_next = prep_hn(0, rows1[0], 0)
        for g in range(gpc):
            hn_cur = hn_next
            if pend is not None:
                graph_back(1, pend, s1f_p, qt1[1])
            if g + 1 < gpc:
                hn_next = prep_hn(g + 1, rows1[(g + 1) // GB],
                                  (g + 1) % GB)
            s1f, _ = graph_front(1, g, rows1[g // GB], g % GB, hn_cur)
            pend, s1f_p = g, s1f
        graph_back(1, pend, s1f_p, qt1[1])

        # ---- final predictor ----
        hgs_bf = pwork.tile([128, gpc], BF16, name="hgs_bf", tag="uf")
        nc.gpsimd.tensor_copy(hgs_bf[:], HGS[:])
        if dbg:
            nc.sync.dma_start(io["d_hgs"][:], hgs_bf[:])
        z1_ps = ps_mlp.tile([128, gpc], F32, name="z1_ps", tag="mlp")
        nc.tensor.matmul(z1_ps[:], wp1_s[:], hgs_bf[:], start=True,
                         stop=True)
        r1 = pwork.tile([128, gpc], BF16, name="r1", tag="uf")
        nc.scalar.activation(r1[:], z1_ps[:], AF.Relu, bias=bp1c_s[:])
        y_ps = ps_mlp.tile([1, gpc], F32, name="y_ps", tag="mlp")
        nc.tensor.matmul(y_ps[:], wp2_s[:], r1[:], start=True, stop=True)
        y_sb = prow.tile([1, gpc], F32, name="y_sb", tag="ysb")
        nc.scalar.activation(y_sb[:], y_ps[:], AF.Identity, bias=bp2c_s[:])
        nc.sync.dma_start(io["y"][:], y_sb[:])


def _build_ct_np(src, dst, npb, nblocks):
    blk = src // npb
    s = src - blk * npb
    d = dst - blk * npb
    flat = blk * (npb * npb) + s * npb + d
    cnt = np.bincount(flat, minlength=nblocks * npb * npb)
    return cnt.reshape(nblocks, npb, npb)


_PROG_CACHE = {}
_PROG_LOCK = threading.Lock()


def _get_program(gpc=GPC):
    with _PROG_LOCK:
        if gpc not in _PROG_CACHE:
            _PROG_CACHE[gpc] = build_program(gpc)
        return _PROG_CACHE[gpc]


def _make_in_maps(inputs, gpc=GPC, ncores=NCORES):
    bf = ml_dtypes.bfloat16
    X = np.asarray(inputs["X"], np.float32)
    X_q = np.asarray(inputs["X_q"], np.float32)
    g_src = np.asarray(inputs["g_src"], np.int64)
    g_dst = np.asarray(inputs["g_dst"], np.int64)
    q_src = np.asarray(inputs["q_src"], np.int64)
    q_dst = np.asarray(inputs["q_dst"], np.int64)

    W1r = np.asarray(inputs["W1r"], np.float32)
    seg16 = np.zeros((128, NCH, G32), np.float32)
    for c in range(NCH):
        for n in range(128):
            seg16[n, c, (c * 128 + n) // NQPG] = 1.0
    shared = {
        "wg": np.asarray(inputs["Wg"], np.float32).astype(bf),
        "wq": np.asarray(inputs["Wq"], np.float32).astype(bf),
        "bgc": np.asarray(inputs["bg"], np.float32).reshape(H, 1).copy(),
        "bqc": np.asarray(inputs["bq"], np.float32).reshape(H, 1).copy(),
        "betg": np.tile(
            np.asarray(inputs["betas_g"], np.float32).reshape(L, 1, 1),
            (1, H, 1)),
        "betq": np.tile(
            np.asarray(inputs["betas_q"], np.float32).reshape(L, 1, 1),
            (1, H, 1)),
        "a1": np.ascontiguousarray(W1r[:, :H, :]).astype(bf),
        "b1t": np.ascontiguousarray(W1r[:, H:, :]).astype(bf),
        "w2": np.asarray(inputs["W2r"], np.float32).astype(bf),
        "b1c": np.asarray(inputs["b1r"], np.float32).reshape(L, H, 1).copy(),
        "b2c": np.asarray(inputs["b2r"], np.float32).reshape(L, H, 1).copy(),
        "wp1": np.asarray(inputs["Wp1"], np.float32).astype(bf),
        "wp2": np.asarray(inputs["Wp2"], np.float32).astype(bf),
        "bp1c": np.asarray(inputs["bp1"], np.float32).reshape(H, 1).copy(),
        "bp2c": np.asarray(inputs["bp2"], np.float32).reshape(1, 1).copy(),
        "seg16": seg16.reshape(128, NCH * G32).astype(bf),
    }

    n = gpc * NPG
    nq = gpc * NQPG
    ne = n * 8
    nqe = nq * 8
    in_maps = []
    for c in range(ncores):
        xc = X[c * n:(c + 1) * n]
        xqc = X_q[c * nq:(c + 1) * nq]
        gs = g_src[c * ne:(c + 1) * ne] - c * n
        gd = g_dst[c * ne:(c + 1) * ne] - c * n
        qs = q_src[c * nqe:(c + 1) * nqe] - c * nq
        qd = q_dst[c * nqe:(c + 1) * nqe] - c * nq

        ct_g = _build_ct_np(gs, gd, NPG, gpc)       # [gpc, 512, 512]
        ct_q = _build_ct_np(qs, qd, NQPG, gpc)      # [gpc, 16, 16]
        ctq_blk = np.zeros((512, 512), np.int64)
        for g in range(gpc):
            ctq_blk[g * NQPG:(g + 1) * NQPG,
                    g * NQPG:(g + 1) * NQPG] = ct_q[g]

        ct_all = np.concatenate([ct_g, ctq_blk[None]], 0)
        ct_all = ct_all.reshape(gpc + 1, NCH, 128, N).astype(bf)
        maskr = (ct_g.sum(axis=1) > 0).astype(np.float32)  # [gpc, 512]

        m = dict(shared)
        m["xt"] = np.ascontiguousarray(xc.T).astype(bf)
        xqt = np.zeros((IN, 512), np.float32)
        xqt[:, :nq] = xqc.T
        m["xqt"] = xqt.astype(bf)
        m["ct"] = ct_all
        m["maskr"] = maskr.reshape(1, gpc * N).astype(bf)
        in_maps.append(m)
    return in_maps


def run(inputs, trace=False, gpc=GPC):
    nc = _get_program(gpc)
    in_maps = _make_in_maps(inputs, gpc=gpc)
    res = run_bass_kernel_spmd(nc, in_maps, list(range(NCORES)), trace=trace)
    ys = [res.results[c]["y"].reshape(-1) for c in range(NCORES)]
    out = np.concatenate(ys).astype(np.float32).reshape(B, OUT)
    return out, res


def kernel(**inputs) -> np.ndarray:
    out, _ = run(inputs, trace=False)
    return out
